# revision 5
# baseline (speedup 1.0000x reference)
"""Trainium2 Bass kernel for nn_DeTree (NODE-style oblivious decision ensemble).

Strategy (tree-sharded over 8 cores, 64 trees/core, full batch per core):
  v2 layout: (tree,depth) on partitions, batch on the free axis.
  1. PE: fv = ecw^T @ x^T where ecw = exp(feat_attention) (softmax numerator),
     plus a ones-column matmul for the softmax denominator Z.
  2. DVE/GPSIMD: bins = clamp(A*fv + B) with per-partition A = 0.5*exp(-lt)/Z,
     B = 0.5 - 0.5*thr*exp(-lt)  (folds softmax normalization, threshold,
     temperature, and the 0.5 t + 0.5 sparsemoid affine into one op),
     gates tile pg = [bins ; 1-bins] blocks.
  3. ACT: glog = ln(pg).
  4. PE: leaf log-sums via a constant 0/1 selection matmul built host-side
     from path_map (2 trees per matmul, K = 8-tree gate window).
  5. ACT: rw = exp(leaf sums)  == prod of gated bins per leaf.
  6. PE: out[t*3+r, b] accumulated over 16 tree-pairs per psum group via
     zero-column response block-diagonal weights.
All matmul operands are float32r (FP22 single-pass PE mode).
"""
import numpy as np
from contextlib import ExitStack

import concourse.bass as bass
import concourse.bacc as bacc
import concourse.tile as tile
import concourse.mybir as mybir
from concourse.bass_utils import run_bass_kernel_spmd

F32 = mybir.dt.float32
F32R = mybir.dt.float32r
AF = mybir.ActivationFunctionType
ALU = mybir.AluOpType

B = 1024          # batch
F = 512           # in_features
T = 512           # num_trees
D = 6             # depth
R = 3             # response_dim
NLEAF = 64
NCORES = 8
T_C = T // NCORES          # 64 trees per core
TPG = 8                    # trees per gate-tile group
NG = T_C // TPG            # 8 groups per core
MROW = 64                  # padded rows per fv M-tile (48 real + 16 pad)
NPAIR = T_C // 2           # 32 tree-pairs per core
PAIRS_PER_EG = 16          # pairs per einsum2 psum accumulation group
EPS = 2.0 ** -20
NH = 2                     # N halves (1024 = 2 x 512)

_CACHE = {}


def _build_sel(path_map):
    """Four [128,128] 0/1 selection matrices (pair k within an 8-tree tile).

    pg tile rows: 6*t_loc+d (bins, rows 0..47), 64 + 6*t_loc+d (1-bins).
    Column 64*t01 + leaf; entry = multiplicity of that gate in the leaf path.
    """
    pm = np.asarray(path_map).reshape(NLEAF, D)
    sel = np.zeros((4, 128, 128), np.float32)
    for k in range(4):
        for t01 in range(2):
            t_loc = 2 * k + t01
            for leaf in range(NLEAF):
                col = 64 * t01 + leaf
                for j in range(D):
                    g = int(pm[leaf, j])
                    d, s = g // 2, g % 2
                    sel[k, (64 if s else 0) + 6 * t_loc + d, col] += 1.0
    # ship as [128, 512]: pair k occupies cols 128k..128k+127
    return np.ascontiguousarray(sel.transpose(1, 0, 2).reshape(128, 512))


def _build_rbd(response_core):
    """[128, 32*96] response block-diagonal lhsT, zero-padded columns.

    Pair p (local to core) -> cols 96p..96p+95; within those, only cols
    6q..6q+5 are nonzero (q = p % 16): row 64*t01 + leaf holds
    response[2p+t01, leaf, r] at col 6q + 3*t01 + r.
    """
    rbd = np.zeros((128, NPAIR * 96), np.float32)
    for p in range(NPAIR):
        q = p % PAIRS_PER_EG
        for t01 in range(2):
            t = 2 * p + t01
            blk = response_core[t]  # [64, 3]
            rbd[64 * t01:64 * t01 + 64, 96 * p + 6 * q + 3 * t01: 96 * p + 6 * q + 3 * t01 + 3] = blk
    return rbd


def _build_program():
    nc = bacc.Bacc("TRN2", target_bir_lowering=False, debug=False,
                   num_devices=NCORES)

    xt = nc.dram_tensor("xt", [F, B], F32R, kind="ExternalInput")
    fap = nc.dram_tensor("fap", [F, NG * MROW], F32, kind="ExternalInput")
    ta0 = nc.dram_tensor("ta0", [MROW, NG], F32, kind="ExternalInput")
    tbb = nc.dram_tensor("tbb", [MROW, NG], F32, kind="ExternalInput")
    ones = nc.dram_tensor("ones", [128, 2], F32R, kind="ExternalInput")
    selz = nc.dram_tensor("selz", [128, 512], F32R, kind="ExternalInput")
    rbd = nc.dram_tensor("rbd", [128, NPAIR * 96], F32R, kind="ExternalInput")
    out = nc.dram_tensor("out", [T_C * R, B], F32, kind="ExternalOutput")

    with tile.TileContext(nc) as tc, ExitStack() as ctx:
        cpool = ctx.enter_context(tc.tile_pool(name="consts", bufs=1))

        txt = [cpool.tile([128, B], F32R, name=f"txt{k}", tag=f"xt{k}") for k in range(4)]
        tfap = [cpool.tile([128, NG * MROW], F32, name=f"tfap{k}", tag=f"fap{k}") for k in range(4)]
        tecw = [cpool.tile([128, NG * MROW], F32R, name=f"tecw{k}", tag=f"ecw{k}") for k in range(4)]
        tta0 = cpool.tile([MROW, NG], F32)
        ttb = cpool.tile([MROW, NG], F32)
        tones = cpool.tile([128, 2], F32R)
        tselz = cpool.tile([128, 512], F32R)
        trbd = cpool.tile([128, NPAIR * 96], F32R)
        tra = cpool.tile([MROW, NG], F32)   # A scalars
        trz = cpool.tile([MROW, 2 * NG], F32)   # 1/Z

        for k in range(4):
            nc.sync.dma_start(txt[k][:], xt[128 * k:128 * k + 128, :])
            nc.sync.dma_start(tfap[k][:], fap[128 * k:128 * k + 128, :])
        nc.sync.dma_start(tta0[:], ta0[:])
        nc.sync.dma_start(ttb[:], tbb[:])
        nc.sync.dma_start(tones[:], ones[:])
        nc.sync.dma_start(tselz[:], selz[:])
        nc.sync.dma_start(trbd[:], rbd[:])

        # ecw = exp(feat_attention), padded cols exp(0)=1
        for k in range(4):
            nc.scalar.activation(tecw[k][:], tfap[k][:], AF.Exp)

        # softmax denominators: Z[row, g] via ones-matmuls
        with tc.tile_pool(name="zps", bufs=1, space="PSUM") as zpool:
            zp = zpool.tile([MROW, 2 * NG], F32)
            for g in range(NG):
                for k in range(4):
                    nc.tensor.matmul(zp[:, 2 * g:2 * g + 2],
                                     tecw[k][:, MROW * g:MROW * (g + 1)],
                                     tones[:],
                                     start=(k == 0), stop=(k == 3))
            nc.vector.reciprocal(trz[:], zp[:])
        nc.vector.tensor_mul(tra[:], tta0[:], trz[:, 0::2])

        with (
            tc.tile_pool(name="fvps", bufs=1, space="PSUM") as fvpool,
            tc.tile_pool(name="sps", bufs=2, space="PSUM") as spool,
            tc.tile_pool(name="ops", bufs=1, space="PSUM") as opool,
            tc.tile_pool(name="work", bufs=2) as wpool,
            tc.tile_pool(name="gates", bufs=3) as gpool,
        ):
            oend = [None, None]
            for eg in range(2):
                op = opool.tile([96, B], F32, name=f"op{eg}", tag="outp")
                for gi in range(NG // 2):
                    g = eg * (NG // 2) + gi
                    # feature-value matmuls
                    fv = fvpool.tile([MROW, B], F32, name=f"fv{g}", tag="fv")
                    for nh in range(NH):
                        for k in range(4):
                            nc.tensor.matmul(
                                fv[:, 512 * nh:512 * (nh + 1)],
                                tecw[k][:, MROW * g:MROW * (g + 1)],
                                txt[k][:, 512 * nh:512 * (nh + 1)],
                                start=(k == 0), stop=(k == 3))
                    # bins / gates
                    tmp = wpool.tile([MROW, B], F32, name=f"tmp{g}", tag="tmp")
                    nc.vector.tensor_scalar(tmp[:], fv[:], tra[:, g:g + 1],
                                            ttb[:, g:g + 1], ALU.mult, ALU.add)
                    pg = gpool.tile([128, B], F32R, name=f"pg{g}", tag="pg")
                    nc.gpsimd.tensor_scalar(pg[0:64, :], tmp[:], 1.0, EPS,
                                            ALU.min, ALU.max)
                    nc.gpsimd.tensor_scalar(pg[64:128, :], tmp[:], -1.0, 1.0,
                                            ALU.mult, ALU.add)
                    nc.gpsimd.tensor_scalar(pg[64:128, :], pg[64:128, :],
                                            1.0 - EPS, EPS, ALU.min, ALU.max)
                    glog = gpool.tile([128, B], F32R, name=f"glog{g}", tag="glog")
                    nc.scalar.activation(glog[:], pg[:], AF.Ln)
                    # per-pair: leaf log-sums -> exp -> response contraction
                    for k in range(4):
                        p = 4 * g + k
                        q = p % PAIRS_PER_EG
                        sp = spool.tile([128, B], F32, name=f"sp{p}", tag="s")
                        for nh in range(NH):
                            nc.tensor.matmul(
                                sp[:, 512 * nh:512 * (nh + 1)],
                                tselz[:, 128 * k:128 * (k + 1)],
                                glog[:, 512 * nh:512 * (nh + 1)],
                                start=True, stop=True)
                        rw = gpool.tile([128, B], F32R, name=f"rw{p}", tag="rw")
                        nc.scalar.activation(rw[:], sp[:], AF.Exp)
                        for nh in range(NH):
                            mm = nc.tensor.matmul(
                                op[:, 512 * nh:512 * (nh + 1)],
                                trbd[:, 96 * p:96 * (p + 1)],
                                rw[:, 512 * nh:512 * (nh + 1)],
                                start=(q == 0), stop=(q == PAIRS_PER_EG - 1),
                                skip_group_check=True)
                # evacuate einsum2 psum group
                ev = wpool.tile([96, B], F32, name=f"ev{eg}", tag="ev")
                nc.vector.tensor_copy(ev[:], op[:])
                nc.sync.dma_start(out[96 * eg:96 * (eg + 1), :], ev[:])

    nc.compile()
    return nc


def kernel(x, feat_attention, thresholds, log_temperatures, response, path_map):
    x = np.ascontiguousarray(np.asarray(x, dtype=np.float32))
    feat_attention = np.asarray(feat_attention, dtype=np.float32)
    thresholds = np.asarray(thresholds, dtype=np.float32)
    log_temperatures = np.asarray(log_temperatures, dtype=np.float32)
    response = np.asarray(response, dtype=np.float32)

    if "nc" not in _CACHE:
        _CACHE["nc"] = _build_program()
    nc = _CACHE["nc"]

    xt = np.ascontiguousarray(x.T)                      # [512, 1024]
    selz = _build_sel(path_map)
    ones = np.ones((128, 2), np.float32)
    elt = np.exp(-log_temperatures)                     # [512, 6]
    a0_all = 0.5 * elt                                  # pre-Z A numerator
    b_all = 0.5 - 0.5 * thresholds * elt

    in_maps = []
    for c in range(NCORES):
        t0 = T_C * c
        fa_c = feat_attention[:, D * t0: D * (t0 + T_C)]   # [512, 384]
        fap = np.zeros((F, NG * MROW), np.float32)
        ta0 = np.zeros((MROW, NG), np.float32)
        tbb = np.full((MROW, NG), 0.5, np.float32)
        for g in range(NG):
            fap[:, MROW * g: MROW * g + 48] = fa_c[:, 48 * g: 48 * g + 48]
            for t_loc in range(TPG):
                t = t0 + TPG * g + t_loc
                rows = slice(6 * t_loc, 6 * t_loc + 6)
                ta0[rows, g] = a0_all[t]
                tbb[rows, g] = b_all[t]
        rbd = _build_rbd(response[t0:t0 + T_C])
        in_maps.append({
            "xt": xt, "fap": fap, "ta0": ta0, "tbb": tbb,
            "ones": ones, "selz": selz, "rbd": rbd,
        })

    _CACHE["in_maps"] = in_maps
    res = run_bass_kernel_spmd(nc, in_maps, core_ids=list(range(NCORES)))
    outs = [res.results[c]["out"].T for c in range(NCORES)]  # [1024, 192] each
    return np.ascontiguousarray(np.concatenate(outs, axis=1))


# revision 7
# speedup vs baseline: 1.1631x; 1.1631x over previous
"""Trainium2 Bass kernel for nn_DeTree (NODE-style oblivious decision ensemble).

Tree-sharded over 8 cores (64 trees/core), full batch per core, layout
[(tree,depth) partitions x batch free].

Fast path (oblivious path_map, leaf bit-split 4+2):
  1. PE: fv = ecw^T @ x^T (ecw = exp(feat_attention)) + ones-column Z matmul.
  2. DVE/GPSIMD: bins = clamp(A*fv + B); gates tile pg = [bins ; 1-bins].
  3. ACT: glog = ln(pg).
  4. PE: lo-sums S2 (16 combos/tree) and replicated hi-sums S1r
     (3r x 4 combos/tree) via constant 0/1 selection matmuls.
  5. ACT: E2 = exp(S2), E1r = exp(S1r).
  6. PE: M1[t,(r,hi)] = sum_lo resp[t,hi*16+lo,r] * E2[t,lo]  (block-diag).
  7. DVE: P = M1 * E1r.
  8. PE: out[t*3+r] = sum_hi P, accumulated 4 groups per psum via
     zero-column selection weights.
Generic path (any path_map): 2-trees-per-matmul leaf log-sum (64 leaves),
exp, response block-diag accumulation.
All matmul operands are float32r (FP22 single-pass PE mode).
"""
import numpy as np
from contextlib import ExitStack

import concourse.bass as bass
import concourse.bacc as bacc
import concourse.tile as tile
import concourse.mybir as mybir
from concourse.bass_utils import run_bass_kernel_spmd

F32 = mybir.dt.float32
F32R = mybir.dt.float32r
AF = mybir.ActivationFunctionType
ALU = mybir.AluOpType

B = 1024          # batch
F = 512           # in_features
T = 512           # num_trees
D = 6             # depth
R = 3             # response_dim
NLEAF = 64
NCORES = 8
T_C = T // NCORES          # 64 trees per core
TPG = 8                    # trees per gate-tile group
NG = T_C // TPG            # 8 groups per core
MROW = 64                  # padded rows per fv M-tile (48 real + 16 pad)
NPAIR = T_C // 2           # generic path: 32 tree-pairs per core
PAIRS_PER_EG = 16
EPS = 2.0 ** -20
NH = 2                     # N halves (1024 = 2 x 512)
NLO = 16                   # 2^4 lo-combos (depths 0..3)
NHI = 4                    # 2^2 hi-combos (depths 4..5)

_CACHE = {}


def _is_oblivious(path_map):
    pm = np.asarray(path_map).reshape(NLEAF, D)
    exp = np.array([[2 * j + ((l >> j) & 1) for j in range(D)]
                    for l in range(NLEAF)], dtype=pm.dtype)
    return bool(np.array_equal(pm, exp))


def _gate_row(t_loc, g):
    """pg-tile row of gate g (= 2d+s) for local tree t_loc."""
    d, s = g // 2, g % 2
    return (64 if s else 0) + 6 * t_loc + d


# ───────────────────────── fast (oblivious) constants ─────────────────────

def _build_sel2c():
    """[128, 128] lo-sum selection: col = 16*t_loc + lo, depths 0..3."""
    S = np.zeros((128, 128), np.float32)
    for t_loc in range(TPG):
        for lo in range(NLO):
            col = NLO * t_loc + lo
            for j in range(4):
                S[_gate_row(t_loc, 2 * j + ((lo >> j) & 1)), col] = 1.0
    return S


def _build_sel1r():
    """[128, 96] replicated hi-sum selection: col = 12*t_loc + 4*r + hi."""
    S = np.zeros((128, 96), np.float32)
    for t_loc in range(TPG):
        for r in range(R):
            for hi in range(NHI):
                col = 12 * t_loc + 4 * r + hi
                for j in range(4, 6):
                    S[_gate_row(t_loc, 2 * j + ((hi >> (j - 4)) & 1)), col] = 1.0
    return S


def _build_selh():
    """[96, 4*96] hi-reduce: 4 variants (group slot in psum accumulation).

    variant v: rows = P rows (12*t_loc + 4*r + hi), col = 24*v + 3*t_loc + r.
    """
    S = np.zeros((96, 4 * 96), np.float32)
    for v in range(4):
        for t_loc in range(TPG):
            for r in range(R):
                for hi in range(NHI):
                    S[12 * t_loc + 4 * r + hi, 96 * v + 24 * v + 3 * t_loc + r] = 1.0
    return S


def _build_resp2(response_core):
    """[128, NG*96]: per group g, rows 16*t_loc+lo, col 12*t_loc+4*r+hi =
    response[8g+t_loc, hi*16+lo, r]."""
    out = np.zeros((128, NG * 96), np.float32)
    for g in range(NG):
        for t_loc in range(TPG):
            t = TPG * g + t_loc
            for hi in range(NHI):
                for r in range(R):
                    out[NLO * t_loc:NLO * t_loc + NLO,
                        96 * g + 12 * t_loc + 4 * r + hi] = \
                        response_core[t, hi * NLO:(hi + 1) * NLO, r]
    return out


# ───────────────────────── generic-path constants ─────────────────────────

def _build_sel_generic(path_map):
    pm = np.asarray(path_map).reshape(NLEAF, D)
    sel = np.zeros((4, 128, 128), np.float32)
    for k in range(4):
        for t01 in range(2):
            t_loc = 2 * k + t01
            for leaf in range(NLEAF):
                col = 64 * t01 + leaf
                for j in range(D):
                    sel[k, _gate_row(t_loc, int(pm[leaf, j])), col] += 1.0
    return np.ascontiguousarray(sel.transpose(1, 0, 2).reshape(128, 512))


def _build_rbd_generic(response_core):
    rbd = np.zeros((128, NPAIR * 96), np.float32)
    for p in range(NPAIR):
        q = p % PAIRS_PER_EG
        for t01 in range(2):
            t = 2 * p + t01
            c0 = 96 * p + 6 * q + 3 * t01
            rbd[64 * t01:64 * t01 + 64, c0:c0 + 3] = response_core[t]
    return rbd


# ───────────────────────── program builders ──────────────────────────────

def _patched_act_tables():
    """Force Ln+Exp onto the shared natural_log_exp_and_others table set
    so the ACT LUT isn't reloaded between ln and exp phases."""
    import concourse.bacc as bacc_mod
    from concourse.hw_specs import get_activation_tables as orig

    def patched(arch):
        tabs = orig(arch)
        if "natural_log_exp_and_others" in tabs:
            for name, funcs in tabs.items():
                if name != "natural_log_exp_and_others":
                    funcs.discard(AF.Ln)
                    funcs.discard(AF.Exp)
        return tabs

    class _Ctx:
        def __enter__(self):
            self.saved = bacc_mod.get_activation_tables
            bacc_mod.get_activation_tables = patched

        def __exit__(self, *a):
            bacc_mod.get_activation_tables = self.saved

    return _Ctx()


def _common_frontend(nc, tc, ctx):
    """DMA inputs, ecw=exp(fa), Z matmuls, A scalars, and per-group
    generator yielding (g, glog_tile). Returns (consts, glog_fn)."""
    xt = nc.dram_tensor("xt", [F, B], F32R, kind="ExternalInput")
    fap = nc.dram_tensor("fap", [F, NG * MROW], F32, kind="ExternalInput")
    ta0 = nc.dram_tensor("ta0", [MROW, NG], F32, kind="ExternalInput")
    tbb = nc.dram_tensor("tbb", [MROW, NG], F32, kind="ExternalInput")
    ones = nc.dram_tensor("ones", [128, 2], F32R, kind="ExternalInput")

    cpool = ctx.enter_context(tc.tile_pool(name="consts", bufs=1))
    txt = [cpool.tile([128, B], F32R, name=f"txt{k}", tag=f"xt{k}") for k in range(4)]
    tfap = [cpool.tile([128, NG * MROW], F32, name=f"tfap{k}", tag=f"fap{k}")
            for k in range(4)]
    tecw = [cpool.tile([128, NG * MROW], F32R, name=f"tecw{k}", tag=f"ecw{k}")
            for k in range(4)]
    tta0 = cpool.tile([MROW, NG], F32)
    ttb = cpool.tile([MROW, NG], F32)
    tones = cpool.tile([128, 2], F32R)
    tra = cpool.tile([MROW, NG], F32)
    trz = cpool.tile([MROW, 2 * NG], F32)

    for k in range(4):
        nc.sync.dma_start(txt[k][:], xt[128 * k:128 * k + 128, :])
        nc.sync.dma_start(tfap[k][:], fap[128 * k:128 * k + 128, :])
    nc.sync.dma_start(tta0[:], ta0[:])
    nc.sync.dma_start(ttb[:], tbb[:])
    nc.sync.dma_start(tones[:], ones[:])

    for k in range(4):
        nc.scalar.activation(tecw[k][:], tfap[k][:], AF.Exp)

    with tc.tile_pool(name="zps", bufs=1, space="PSUM") as zpool:
        zp = zpool.tile([MROW, 2 * NG], F32)
        for g in range(NG):
            for k in range(4):
                nc.tensor.matmul(zp[:, 2 * g:2 * g + 2],
                                 tecw[k][:, MROW * g:MROW * (g + 1)],
                                 tones[:], start=(k == 0), stop=(k == 3))
        nc.vector.reciprocal(trz[:], zp[:])
    nc.vector.tensor_mul(tra[:], tta0[:], trz[:, 0::2])

    return cpool, txt, tecw, tra, ttb


def _emit_glogs(nc, tc, ctx, txt, tecw, tra, ttb, glog_bufs):
    """Per-group gate-log tiles. Returns list of glog APs (all live)."""
    fvpool = ctx.enter_context(tc.tile_pool(name="fvps", bufs=1, space="PSUM"))
    wpool = ctx.enter_context(tc.tile_pool(name="work", bufs=2))
    pgpool = ctx.enter_context(tc.tile_pool(name="pgp", bufs=2))
    glpool = ctx.enter_context(tc.tile_pool(name="glp", bufs=glog_bufs))
    glogs = []
    for g in range(NG):
        fv = fvpool.tile([MROW, B], F32, name=f"fv{g}", tag="fv")
        for k in range(4):
            for nh in range(NH):
                nc.tensor.matmul(fv[:, 512 * nh:512 * (nh + 1)],
                                 tecw[k][:, MROW * g:MROW * (g + 1)],
                                 txt[k][:, 512 * nh:512 * (nh + 1)],
                                 start=(k == 0), stop=(k == 3))
        tmp = wpool.tile([MROW, B], F32, name=f"tmp{g}", tag="tmp")
        nc.vector.tensor_scalar(tmp[:], fv[:], tra[:, g:g + 1],
                                ttb[:, g:g + 1], ALU.mult, ALU.add)
        pg = pgpool.tile([128, B], F32R, name=f"pg{g}", tag="pg")
        nc.gpsimd.tensor_scalar(pg[0:64, :], tmp[:], 1.0, EPS, ALU.min, ALU.max)
        nc.gpsimd.tensor_scalar(pg[64:128, :], tmp[:], -1.0, 1.0, ALU.mult, ALU.add)
        nc.gpsimd.tensor_scalar(pg[64:128, :], pg[64:128, :], 1.0 - EPS, EPS,
                                ALU.min, ALU.max)
        glog = glpool.tile([128, B], F32R, name=f"glog{g}", tag="glog")
        nc.scalar.activation(glog[:], pg[:], AF.Ln)
        glogs.append(glog)
    return glogs


def _build_program_fast():
    nc = bacc.Bacc("TRN2", target_bir_lowering=False, debug=False,
                   num_devices=NCORES)
    sel2c = nc.dram_tensor("sel2c", [128, 128], F32R, kind="ExternalInput")
    sel1r = nc.dram_tensor("sel1r", [128, 96], F32R, kind="ExternalInput")
    selh = nc.dram_tensor("selh", [96, 4 * 96], F32R, kind="ExternalInput")
    resp2 = nc.dram_tensor("resp2", [128, NG * 96], F32R, kind="ExternalInput")
    out = nc.dram_tensor("out", [T_C * R, B], F32, kind="ExternalOutput")

    with tile.TileContext(nc) as tc, ExitStack() as ctx:
        cpool, txt, tecw, tra, ttb = _common_frontend(nc, tc, ctx)
        tsel2c = cpool.tile([128, 128], F32R)
        tsel1r = cpool.tile([128, 96], F32R)
        tselh = cpool.tile([96, 4 * 96], F32R)
        tresp2 = cpool.tile([128, NG * 96], F32R)
        nc.sync.dma_start(tsel2c[:], sel2c[:])
        nc.sync.dma_start(tsel1r[:], sel1r[:])
        nc.sync.dma_start(tselh[:], selh[:])
        nc.sync.dma_start(tresp2[:], resp2[:])

        glogs = _emit_glogs(nc, tc, ctx, txt, tecw, tra, ttb, glog_bufs=NG)

        e2pool = ctx.enter_context(tc.tile_pool(name="e2p", bufs=NG))
        e1pool = ctx.enter_context(tc.tile_pool(name="e1p", bufs=NG))
        ppool = ctx.enter_context(tc.tile_pool(name="pp", bufs=3))
        evpool = ctx.enter_context(tc.tile_pool(name="evp", bufs=2))

        # lo-sums + exp (stationary sel2c held across all groups)
        e2s, e1s = [], []
        with tc.tile_pool(name="s2ps", bufs=2, space="PSUM") as s2pool:
            for g in range(NG):
                s2 = s2pool.tile([128, B], F32, name=f"s2_{g}", tag="s2")
                for nh in range(NH):
                    nc.tensor.matmul(s2[:, 512 * nh:512 * (nh + 1)], tsel2c[:],
                                     glogs[g][:, 512 * nh:512 * (nh + 1)],
                                     start=True, stop=True)
                e2 = e2pool.tile([128, B], F32R, name=f"e2_{g}", tag="e2")
                nc.scalar.activation(e2[:], s2[:], AF.Exp)
                e2s.append(e2)
        # hi-sums + exp
        with tc.tile_pool(name="s1ps", bufs=2, space="PSUM") as s1pool:
            for g in range(NG):
                s1 = s1pool.tile([96, B], F32, name=f"s1_{g}", tag="s1")
                for nh in range(NH):
                    nc.tensor.matmul(s1[:, 512 * nh:512 * (nh + 1)], tsel1r[:],
                                     glogs[g][:, 512 * nh:512 * (nh + 1)],
                                     start=True, stop=True)
                e1 = e1pool.tile([96, B], F32R, name=f"e1_{g}", tag="e1")
                nc.scalar.activation(e1[:], s1[:], AF.Exp)
                e1s.append(e1)
        # response contraction + hi reduce
        with (
            tc.tile_pool(name="m1ps", bufs=2, space="PSUM") as m1pool,
            tc.tile_pool(name="ops", bufs=1, space="PSUM") as opool,
        ):
            for eg in range(2):
                op = opool.tile([96, B], F32, name=f"op{eg}", tag="outp")
                for v in range(4):
                    g = 4 * eg + v
                    m1 = m1pool.tile([96, B], F32, name=f"m1_{g}", tag="m1")
                    for nh in range(NH):
                        nc.tensor.matmul(m1[:, 512 * nh:512 * (nh + 1)],
                                         tresp2[:, 96 * g:96 * (g + 1)],
                                         e2s[g][:, 512 * nh:512 * (nh + 1)],
                                         start=True, stop=True)
                    pp = ppool.tile([96, B], F32R, name=f"pp{g}", tag="pp")
                    nc.vector.tensor_mul(pp[:], m1[:], e1s[g][:])
                    for nh in range(NH):
                        nc.tensor.matmul(op[:, 512 * nh:512 * (nh + 1)],
                                         tselh[:, 96 * v:96 * (v + 1)],
                                         pp[:, 512 * nh:512 * (nh + 1)],
                                         start=(v == 0), stop=(v == 3),
                                         skip_group_check=True)
                ev = evpool.tile([96, B], F32, name=f"ev{eg}", tag="ev")
                nc.vector.tensor_copy(ev[:], op[:])
                nc.sync.dma_start(out[96 * eg:96 * (eg + 1), :], ev[:])

    with _patched_act_tables():
        nc.compile()
    return nc


def _build_program_generic():
    nc = bacc.Bacc("TRN2", target_bir_lowering=False, debug=False,
                   num_devices=NCORES)
    selz = nc.dram_tensor("selz", [128, 512], F32R, kind="ExternalInput")
    rbd = nc.dram_tensor("rbd", [128, NPAIR * 96], F32R, kind="ExternalInput")
    out = nc.dram_tensor("out", [T_C * R, B], F32, kind="ExternalOutput")

    with tile.TileContext(nc) as tc, ExitStack() as ctx:
        cpool, txt, tecw, tra, ttb = _common_frontend(nc, tc, ctx)
        tselz = cpool.tile([128, 512], F32R)
        trbd = cpool.tile([128, NPAIR * 96], F32R)
        nc.sync.dma_start(tselz[:], selz[:])
        nc.sync.dma_start(trbd[:], rbd[:])

        glogs = _emit_glogs(nc, tc, ctx, txt, tecw, tra, ttb, glog_bufs=3)

        rwpool = ctx.enter_context(tc.tile_pool(name="rwp", bufs=3))
        evpool = ctx.enter_context(tc.tile_pool(name="evp", bufs=2))
        with (
            tc.tile_pool(name="sps", bufs=2, space="PSUM") as spool,
            tc.tile_pool(name="ops", bufs=1, space="PSUM") as opool,
        ):
            for eg in range(2):
                op = opool.tile([96, B], F32, name=f"op{eg}", tag="outp")
                for gi in range(NG // 2):
                    g = eg * (NG // 2) + gi
                    for k in range(4):
                        p = 4 * g + k
                        q = p % PAIRS_PER_EG
                        sp = spool.tile([128, B], F32, name=f"sp{p}", tag="s")
                        for nh in range(NH):
                            nc.tensor.matmul(sp[:, 512 * nh:512 * (nh + 1)],
                                             tselz[:, 128 * k:128 * (k + 1)],
                                             glogs[g][:, 512 * nh:512 * (nh + 1)],
                                             start=True, stop=True)
                        rw = rwpool.tile([128, B], F32R, name=f"rw{p}", tag="rw")
                        nc.scalar.activation(rw[:], sp[:], AF.Exp)
                        for nh in range(NH):
                            nc.tensor.matmul(op[:, 512 * nh:512 * (nh + 1)],
                                             trbd[:, 96 * p:96 * (p + 1)],
                                             rw[:, 512 * nh:512 * (nh + 1)],
                                             start=(q == 0),
                                             stop=(q == PAIRS_PER_EG - 1),
                                             skip_group_check=True)
                ev = evpool.tile([96, B], F32, name=f"ev{eg}", tag="ev")
                nc.vector.tensor_copy(ev[:], op[:])
                nc.sync.dma_start(out[96 * eg:96 * (eg + 1), :], ev[:])

    with _patched_act_tables():
        nc.compile()
    return nc


# ───────────────────────── host entry point ──────────────────────────────

def _host_prep_core(c, xt, feat_attention, a0_all, b_all):
    t0 = T_C * c
    fa_c = feat_attention[:, D * t0: D * (t0 + T_C)]
    fap = np.zeros((F, NG * MROW), np.float32)
    ta0 = np.zeros((MROW, NG), np.float32)
    tbb = np.full((MROW, NG), 0.5, np.float32)
    for g in range(NG):
        fap[:, MROW * g: MROW * g + 48] = fa_c[:, 48 * g: 48 * g + 48]
        for t_loc in range(TPG):
            t = t0 + TPG * g + t_loc
            rows = slice(6 * t_loc, 6 * t_loc + 6)
            ta0[rows, g] = a0_all[t]
            tbb[rows, g] = b_all[t]
    return dict(xt=xt, fap=fap, ta0=ta0, tbb=tbb,
                ones=np.ones((128, 2), np.float32))


def kernel(x, feat_attention, thresholds, log_temperatures, response, path_map):
    x = np.ascontiguousarray(np.asarray(x, dtype=np.float32))
    feat_attention = np.asarray(feat_attention, dtype=np.float32)
    thresholds = np.asarray(thresholds, dtype=np.float32)
    log_temperatures = np.asarray(log_temperatures, dtype=np.float32)
    response = np.asarray(response, dtype=np.float32)

    fast = _is_oblivious(path_map)
    key = "fast" if fast else "generic"
    if key not in _CACHE:
        _CACHE[key] = (_build_program_fast() if fast
                       else _build_program_generic())
    nc = _CACHE[key]

    xt = np.ascontiguousarray(x.T)
    elt = np.exp(-log_temperatures)
    a0_all = 0.5 * elt
    b_all = 0.5 - 0.5 * thresholds * elt

    in_maps = []
    for c in range(NCORES):
        m = _host_prep_core(c, xt, feat_attention, a0_all, b_all)
        t0 = T_C * c
        if fast:
            m["sel2c"] = _CACHE.setdefault("sel2c", _build_sel2c())
            m["sel1r"] = _CACHE.setdefault("sel1r", _build_sel1r())
            m["selh"] = _CACHE.setdefault("selh", _build_selh())
            m["resp2"] = _build_resp2(response[t0:t0 + T_C])
        else:
            if "selg" not in _CACHE:
                _CACHE["selg"] = _build_sel_generic(path_map)
            m["selz"] = _CACHE["selg"]
            m["rbd"] = _build_rbd_generic(response[t0:t0 + T_C])
        in_maps.append(m)

    _CACHE["in_maps"] = in_maps
    res = run_bass_kernel_spmd(nc, in_maps, core_ids=list(range(NCORES)))
    outs = [res.results[c]["out"].T for c in range(NCORES)]
    return np.ascontiguousarray(np.concatenate(outs, axis=1))


# revision 8
# speedup vs baseline: 1.2614x; 1.0844x over previous
"""Trainium2 Bass kernel for nn_DeTree (NODE-style oblivious decision ensemble).

Tree-sharded over 8 cores (64 trees/core), full batch per core, layout
[(tree,depth) partitions x batch free].

Fast path (oblivious path_map, leaf bit-split 4+2):
  1. PE: fv = ecw^T @ x^T (ecw = exp(feat_attention)) + ones-column Z matmul.
  2. DVE/GPSIMD: bins = clamp(A*fv + B); gates tile pg = [bins ; 1-bins].
  3. ACT: glog = ln(pg).
  4. PE: lo-sums S2 (16 combos/tree) and replicated hi-sums S1r
     (3r x 4 combos/tree) via constant 0/1 selection matmuls.
  5. ACT: E2 = exp(S2), E1r = exp(S1r).
  6. PE: M1[t,(r,hi)] = sum_lo resp[t,hi*16+lo,r] * E2[t,lo]  (block-diag).
  7. DVE: P = M1 * E1r.
  8. PE: out[t*3+r] = sum_hi P, accumulated 4 groups per psum via
     zero-column selection weights.
Generic path (any path_map): 2-trees-per-matmul leaf log-sum (64 leaves),
exp, response block-diag accumulation.
All matmul operands are float32r (FP22 single-pass PE mode).
"""
import numpy as np
from contextlib import ExitStack

import concourse.bass as bass
import concourse.bacc as bacc
import concourse.tile as tile
import concourse.mybir as mybir
from concourse.bass_utils import run_bass_kernel_spmd

F32 = mybir.dt.float32
F32R = mybir.dt.float32r
AF = mybir.ActivationFunctionType
ALU = mybir.AluOpType

B = 1024          # batch
F = 512           # in_features
T = 512           # num_trees
D = 6             # depth
R = 3             # response_dim
NLEAF = 64
NCORES = 8
T_C = T // NCORES          # 64 trees per core
TPG = 8                    # trees per gate-tile group
NG = T_C // TPG            # 8 groups per core
MROW = 64                  # padded rows per fv M-tile (48 real + 16 pad)
NPAIR = T_C // 2           # generic path: 32 tree-pairs per core
PAIRS_PER_EG = 16
EPS = 2.0 ** -20
NH = 2                     # N halves (1024 = 2 x 512)
NLO = 16                   # 2^4 lo-combos (depths 0..3)
NHI = 4                    # 2^2 hi-combos (depths 4..5)

_CACHE = {}


def _is_oblivious(path_map):
    pm = np.asarray(path_map).reshape(NLEAF, D)
    exp = np.array([[2 * j + ((l >> j) & 1) for j in range(D)]
                    for l in range(NLEAF)], dtype=pm.dtype)
    return bool(np.array_equal(pm, exp))


def _gate_row(t_loc, g):
    """pg-tile row of gate g (= 2d+s) for local tree t_loc."""
    d, s = g // 2, g % 2
    return (64 if s else 0) + 6 * t_loc + d


# ───────────────────────── fast (oblivious) constants ─────────────────────

def _build_sel2c():
    """[128, 128] lo-sum selection: col = 16*t_loc + lo, depths 0..3."""
    S = np.zeros((128, 128), np.float32)
    for t_loc in range(TPG):
        for lo in range(NLO):
            col = NLO * t_loc + lo
            for j in range(4):
                S[_gate_row(t_loc, 2 * j + ((lo >> j) & 1)), col] = 1.0
    return S


def _build_sel1r():
    """[128, 96] replicated hi-sum selection: col = 12*t_loc + 4*r + hi."""
    S = np.zeros((128, 96), np.float32)
    for t_loc in range(TPG):
        for r in range(R):
            for hi in range(NHI):
                col = 12 * t_loc + 4 * r + hi
                for j in range(4, 6):
                    S[_gate_row(t_loc, 2 * j + ((hi >> (j - 4)) & 1)), col] = 1.0
    return S


def _build_selh():
    """[96, 4*96] hi-reduce: 4 variants (group slot in psum accumulation).

    variant v: rows = P rows (12*t_loc + 4*r + hi), col = 24*v + 3*t_loc + r.
    """
    S = np.zeros((96, 4 * 96), np.float32)
    for v in range(4):
        for t_loc in range(TPG):
            for r in range(R):
                for hi in range(NHI):
                    S[12 * t_loc + 4 * r + hi, 96 * v + 24 * v + 3 * t_loc + r] = 1.0
    return S


def _build_resp2(response_core):
    """[128, NG*96]: per group g, rows 16*t_loc+lo, col 12*t_loc+4*r+hi =
    response[8g+t_loc, hi*16+lo, r]."""
    out = np.zeros((128, NG * 96), np.float32)
    for g in range(NG):
        for t_loc in range(TPG):
            t = TPG * g + t_loc
            for hi in range(NHI):
                for r in range(R):
                    out[NLO * t_loc:NLO * t_loc + NLO,
                        96 * g + 12 * t_loc + 4 * r + hi] = \
                        response_core[t, hi * NLO:(hi + 1) * NLO, r]
    return out


# ───────────────────────── generic-path constants ─────────────────────────

def _build_sel_generic(path_map):
    pm = np.asarray(path_map).reshape(NLEAF, D)
    sel = np.zeros((4, 128, 128), np.float32)
    for k in range(4):
        for t01 in range(2):
            t_loc = 2 * k + t01
            for leaf in range(NLEAF):
                col = 64 * t01 + leaf
                for j in range(D):
                    sel[k, _gate_row(t_loc, int(pm[leaf, j])), col] += 1.0
    return np.ascontiguousarray(sel.transpose(1, 0, 2).reshape(128, 512))


def _build_rbd_generic(response_core):
    rbd = np.zeros((128, NPAIR * 96), np.float32)
    for p in range(NPAIR):
        q = p % PAIRS_PER_EG
        for t01 in range(2):
            t = 2 * p + t01
            c0 = 96 * p + 6 * q + 3 * t01
            rbd[64 * t01:64 * t01 + 64, c0:c0 + 3] = response_core[t]
    return rbd


# ───────────────────────── program builders ──────────────────────────────

def _patched_act_tables():
    """Force Ln+Exp onto the shared natural_log_exp_and_others table set
    so the ACT LUT isn't reloaded between ln and exp phases."""
    import concourse.bacc as bacc_mod
    from concourse.hw_specs import get_activation_tables as orig

    def patched(arch):
        tabs = orig(arch)
        if "natural_log_exp_and_others" in tabs:
            for name, funcs in tabs.items():
                if name != "natural_log_exp_and_others":
                    funcs.discard(AF.Ln)
                    funcs.discard(AF.Exp)
        return tabs

    class _Ctx:
        def __enter__(self):
            self.saved = bacc_mod.get_activation_tables
            bacc_mod.get_activation_tables = patched

        def __exit__(self, *a):
            bacc_mod.get_activation_tables = self.saved

    return _Ctx()


def _common_frontend(nc, tc, ctx):
    """DMA inputs and ecw = exp(feat_attention) tiles."""
    xt = nc.dram_tensor("xt", [F, B + 2], F32R, kind="ExternalInput")
    fap = nc.dram_tensor("fap", [F, NG * MROW], F32, kind="ExternalInput")
    ta0 = nc.dram_tensor("ta0", [128, 4], F32, kind="ExternalInput")
    tbb = nc.dram_tensor("tbb", [128, 4], F32, kind="ExternalInput")

    cpool = ctx.enter_context(tc.tile_pool(name="consts", bufs=1))
    txt = [cpool.tile([128, B + 2], F32R, name=f"txt{k}", tag=f"xt{k}")
           for k in range(4)]
    tfap = [cpool.tile([128, NG * MROW], F32, name=f"tfap{k}", tag=f"fap{k}")
            for k in range(4)]
    tecw = [cpool.tile([128, NG * MROW], F32R, name=f"tecw{k}", tag=f"ecw{k}")
            for k in range(4)]
    tta0 = cpool.tile([128, 4], F32)
    ttb = cpool.tile([128, 4], F32)
    tra = cpool.tile([128, 4], F32)
    trz = cpool.tile([128, 8], F32)

    for k in range(4):
        nc.sync.dma_start(txt[k][:], xt[128 * k:128 * k + 128, :])
        nc.sync.dma_start(tfap[k][:], fap[128 * k:128 * k + 128, :])
    nc.sync.dma_start(tta0[:], ta0[:])
    nc.sync.dma_start(ttb[:], tbb[:])

    for k in range(4):
        nc.scalar.activation(tecw[k][:], tfap[k][:], AF.Exp)

    return cpool, txt, tecw, tra, trz, tta0, ttb


def _emit_glogs(nc, tc, ctx, txt, tecw, tra, trz, tta0, ttb, glog_bufs):
    """Per-group gate-log tiles via M=128 fv matmuls with fused Z columns.
    Returns list of glog APs."""
    glpool = ctx.enter_context(tc.tile_pool(name="glp", bufs=glog_bufs))
    lctx = ctx.enter_context(ExitStack())
    fvpool = lctx.enter_context(tc.tile_pool(name="fvps", bufs=1, space="PSUM"))
    wpool = lctx.enter_context(tc.tile_pool(name="work", bufs=2))
    pgpool = lctx.enter_context(tc.tile_pool(name="pgp", bufs=2))
    glogs = [None] * NG
    for m in range(4):          # M-tile = 2 gate groups (2m, 2m+1)
        fv = fvpool.tile([128, B + 2], F32, name=f"fv{m}", tag="fv")
        for k in range(4):
            for off, n in ((0, 512), (512, 512), (1024, 2)):
                nc.tensor.matmul(fv[:, off:off + n],
                                 tecw[k][:, 128 * m:128 * (m + 1)],
                                 txt[k][:, off:off + n],
                                 start=(k == 0), stop=(k == 3))
        nc.vector.reciprocal(trz[:, 2 * m:2 * m + 2], fv[:, 1024:1026])
        nc.vector.tensor_mul(tra[:, m:m + 1], tta0[:, m:m + 1],
                             trz[:, 2 * m:2 * m + 1])
        tmp = wpool.tile([128, B], F32, name=f"tmp{m}", tag="tmp")
        nc.vector.tensor_scalar(tmp[:], fv[:, 0:1024], tra[:, m:m + 1],
                                ttb[:, m:m + 1], ALU.mult, ALU.add)
        for half in range(2):
            g = 2 * m + half
            th = tmp[64 * half:64 * half + 64, :]
            pg = pgpool.tile([128, B], F32R, name=f"pg{g}", tag="pg")
            nc.gpsimd.tensor_scalar(pg[0:64, :], th, 1.0, EPS, ALU.min, ALU.max)
            nc.gpsimd.tensor_scalar(pg[64:128, :], th, -1.0, 1.0,
                                    ALU.mult, ALU.add)
            nc.vector.tensor_scalar(pg[64:128, :], pg[64:128, :], 1.0 - EPS,
                                    EPS, ALU.min, ALU.max)
            glog = glpool.tile([128, B], F32R, name=f"glog{g}", tag="glog")
            nc.scalar.activation(glog[:], pg[:], AF.Ln)
            glogs[g] = glog
    lctx.close()
    return glogs


def _build_program_fast():
    nc = bacc.Bacc("TRN2", target_bir_lowering=False, debug=False,
                   num_devices=NCORES)
    sel2c = nc.dram_tensor("sel2c", [128, 128], F32R, kind="ExternalInput")
    sel1r = nc.dram_tensor("sel1r", [128, 96], F32R, kind="ExternalInput")
    selh = nc.dram_tensor("selh", [96, 4 * 96], F32R, kind="ExternalInput")
    resp2 = nc.dram_tensor("resp2", [128, NG * 96], F32R, kind="ExternalInput")
    out = nc.dram_tensor("out", [T_C * R, B], F32, kind="ExternalOutput")

    with tile.TileContext(nc) as tc, ExitStack() as ctx:
        cpool, txt, tecw, tra, trz, tta0, ttb = _common_frontend(nc, tc, ctx)
        tsel2c = cpool.tile([128, 128], F32R)
        tsel1r = cpool.tile([128, 96], F32R)
        tselh = cpool.tile([96, 4 * 96], F32R)
        tresp2 = cpool.tile([128, NG * 96], F32R)
        nc.sync.dma_start(tsel2c[:], sel2c[:])
        nc.sync.dma_start(tsel1r[:], sel1r[:])
        nc.sync.dma_start(tselh[:], selh[:])
        nc.sync.dma_start(tresp2[:], resp2[:])

        glogs = _emit_glogs(nc, tc, ctx, txt, tecw, tra, trz, tta0, ttb,
                            glog_bufs=NG)

        e2pool = ctx.enter_context(tc.tile_pool(name="e2p", bufs=NG))
        e1pool = ctx.enter_context(tc.tile_pool(name="e1p", bufs=NG))
        ppool = ctx.enter_context(tc.tile_pool(name="pp", bufs=3))
        evpool = ctx.enter_context(tc.tile_pool(name="evp", bufs=2))

        # lo-sums + exp (stationary sel2c held across all groups)
        e2s, e1s = [], []
        with tc.tile_pool(name="s2ps", bufs=2, space="PSUM") as s2pool:
            for g in range(NG):
                s2 = s2pool.tile([128, B], F32, name=f"s2_{g}", tag="s2")
                for nh in range(NH):
                    nc.tensor.matmul(s2[:, 512 * nh:512 * (nh + 1)], tsel2c[:],
                                     glogs[g][:, 512 * nh:512 * (nh + 1)],
                                     start=True, stop=True)
                e2 = e2pool.tile([128, B], F32R, name=f"e2_{g}", tag="e2")
                nc.scalar.activation(e2[:], s2[:], AF.Exp)
                e2s.append(e2)
        # hi-sums + exp
        with tc.tile_pool(name="s1ps", bufs=2, space="PSUM") as s1pool:
            for g in range(NG):
                s1 = s1pool.tile([96, B], F32, name=f"s1_{g}", tag="s1")
                for nh in range(NH):
                    nc.tensor.matmul(s1[:, 512 * nh:512 * (nh + 1)], tsel1r[:],
                                     glogs[g][:, 512 * nh:512 * (nh + 1)],
                                     start=True, stop=True)
                e1 = e1pool.tile([96, B], F32R, name=f"e1_{g}", tag="e1")
                nc.scalar.activation(e1[:], s1[:], AF.Exp)
                e1s.append(e1)
        # response contraction + hi reduce
        with (
            tc.tile_pool(name="m1ps", bufs=2, space="PSUM") as m1pool,
            tc.tile_pool(name="ops", bufs=1, space="PSUM") as opool,
        ):
            for eg in range(2):
                op = opool.tile([96, B], F32, name=f"op{eg}", tag="outp")
                for v in range(4):
                    g = 4 * eg + v
                    m1 = m1pool.tile([96, B], F32, name=f"m1_{g}", tag="m1")
                    for nh in range(NH):
                        nc.tensor.matmul(m1[:, 512 * nh:512 * (nh + 1)],
                                         tresp2[:, 96 * g:96 * (g + 1)],
                                         e2s[g][:, 512 * nh:512 * (nh + 1)],
                                         start=True, stop=True)
                    pp = ppool.tile([96, B], F32R, name=f"pp{g}", tag="pp")
                    nc.vector.tensor_mul(pp[:], m1[:], e1s[g][:])
                    for nh in range(NH):
                        nc.tensor.matmul(op[:, 512 * nh:512 * (nh + 1)],
                                         tselh[:, 96 * v:96 * (v + 1)],
                                         pp[:, 512 * nh:512 * (nh + 1)],
                                         start=(v == 0), stop=(v == 3),
                                         skip_group_check=True)
                ev = evpool.tile([96, B], F32, name=f"ev{eg}", tag="ev")
                nc.vector.tensor_copy(ev[:], op[:])
                nc.sync.dma_start(out[96 * eg:96 * (eg + 1), :], ev[:])

    with _patched_act_tables():
        nc.compile()
    return nc


def _build_program_generic():
    nc = bacc.Bacc("TRN2", target_bir_lowering=False, debug=False,
                   num_devices=NCORES)
    selz = nc.dram_tensor("selz", [128, 512], F32R, kind="ExternalInput")
    rbd = nc.dram_tensor("rbd", [128, NPAIR * 96], F32R, kind="ExternalInput")
    out = nc.dram_tensor("out", [T_C * R, B], F32, kind="ExternalOutput")

    with tile.TileContext(nc) as tc, ExitStack() as ctx:
        cpool, txt, tecw, tra, trz, tta0, ttb = _common_frontend(nc, tc, ctx)
        tselz = cpool.tile([128, 512], F32R)
        trbd = cpool.tile([128, NPAIR * 96], F32R)
        nc.sync.dma_start(tselz[:], selz[:])
        nc.sync.dma_start(trbd[:], rbd[:])

        glogs = _emit_glogs(nc, tc, ctx, txt, tecw, tra, trz, tta0, ttb,
                            glog_bufs=3)

        rwpool = ctx.enter_context(tc.tile_pool(name="rwp", bufs=3))
        evpool = ctx.enter_context(tc.tile_pool(name="evp", bufs=2))
        with (
            tc.tile_pool(name="sps", bufs=2, space="PSUM") as spool,
            tc.tile_pool(name="ops", bufs=1, space="PSUM") as opool,
        ):
            for eg in range(2):
                op = opool.tile([96, B], F32, name=f"op{eg}", tag="outp")
                for gi in range(NG // 2):
                    g = eg * (NG // 2) + gi
                    for k in range(4):
                        p = 4 * g + k
                        q = p % PAIRS_PER_EG
                        sp = spool.tile([128, B], F32, name=f"sp{p}", tag="s")
                        for nh in range(NH):
                            nc.tensor.matmul(sp[:, 512 * nh:512 * (nh + 1)],
                                             tselz[:, 128 * k:128 * (k + 1)],
                                             glogs[g][:, 512 * nh:512 * (nh + 1)],
                                             start=True, stop=True)
                        rw = rwpool.tile([128, B], F32R, name=f"rw{p}", tag="rw")
                        nc.scalar.activation(rw[:], sp[:], AF.Exp)
                        for nh in range(NH):
                            nc.tensor.matmul(op[:, 512 * nh:512 * (nh + 1)],
                                             trbd[:, 96 * p:96 * (p + 1)],
                                             rw[:, 512 * nh:512 * (nh + 1)],
                                             start=(q == 0),
                                             stop=(q == PAIRS_PER_EG - 1),
                                             skip_group_check=True)
                ev = evpool.tile([96, B], F32, name=f"ev{eg}", tag="ev")
                nc.vector.tensor_copy(ev[:], op[:])
                nc.sync.dma_start(out[96 * eg:96 * (eg + 1), :], ev[:])

    with _patched_act_tables():
        nc.compile()
    return nc


# ───────────────────────── host entry point ──────────────────────────────

def _host_prep_core(c, xto, feat_attention, a0_all, b_all):
    t0 = T_C * c
    fa_c = feat_attention[:, D * t0: D * (t0 + T_C)]
    fap = np.zeros((F, NG * MROW), np.float32)
    ta0 = np.zeros((128, 4), np.float32)
    tbb = np.full((128, 4), 0.5, np.float32)
    for g in range(NG):
        fap[:, MROW * g: MROW * g + 48] = fa_c[:, 48 * g: 48 * g + 48]
        m, half = g // 2, g % 2
        for t_loc in range(TPG):
            t = t0 + TPG * g + t_loc
            rows = slice(64 * half + 6 * t_loc, 64 * half + 6 * t_loc + 6)
            ta0[rows, m] = a0_all[t]
            tbb[rows, m] = b_all[t]
    return dict(xt=xto, fap=fap, ta0=ta0, tbb=tbb)


def kernel(x, feat_attention, thresholds, log_temperatures, response, path_map):
    x = np.ascontiguousarray(np.asarray(x, dtype=np.float32))
    feat_attention = np.asarray(feat_attention, dtype=np.float32)
    thresholds = np.asarray(thresholds, dtype=np.float32)
    log_temperatures = np.asarray(log_temperatures, dtype=np.float32)
    response = np.asarray(response, dtype=np.float32)

    fast = _is_oblivious(path_map)
    key = "fast" if fast else "generic"
    if key not in _CACHE:
        _CACHE[key] = (_build_program_fast() if fast
                       else _build_program_generic())
    nc = _CACHE[key]

    xto = np.ascontiguousarray(
        np.concatenate([x.T, np.ones((F, 2), np.float32)], axis=1))
    elt = np.exp(-log_temperatures)
    a0_all = 0.5 * elt
    b_all = 0.5 - 0.5 * thresholds * elt

    in_maps = []
    for c in range(NCORES):
        m = _host_prep_core(c, xto, feat_attention, a0_all, b_all)
        t0 = T_C * c
        if fast:
            m["sel2c"] = _CACHE.setdefault("sel2c", _build_sel2c())
            m["sel1r"] = _CACHE.setdefault("sel1r", _build_sel1r())
            m["selh"] = _CACHE.setdefault("selh", _build_selh())
            m["resp2"] = _build_resp2(response[t0:t0 + T_C])
        else:
            if "selg" not in _CACHE:
                _CACHE["selg"] = _build_sel_generic(path_map)
            m["selz"] = _CACHE["selg"]
            m["rbd"] = _build_rbd_generic(response[t0:t0 + T_C])
        in_maps.append(m)

    _CACHE["in_maps"] = in_maps
    res = run_bass_kernel_spmd(nc, in_maps, core_ids=list(range(NCORES)))
    outs = [res.results[c]["out"].T for c in range(NCORES)]
    return np.ascontiguousarray(np.concatenate(outs, axis=1))


# revision 12
# speedup vs baseline: 1.4079x; 1.1162x over previous
"""Trainium2 Bass kernel for nn_DeTree (NODE-style oblivious decision ensemble).

Tree-sharded over 8 cores (64 trees/core), full batch per core, layout
[(tree,depth) partitions x batch free].

Fast path (oblivious path_map, leaf bit-split 4+2):
  1. PE: fv = ecw^T @ x^T (ecw = exp(feat_attention)) + ones-column Z matmul.
  2. DVE/GPSIMD: bins = clamp(A*fv + B); gates tile pg = [bins ; 1-bins].
  3. ACT: glog = ln(pg).
  4. PE: lo-sums S2 (16 combos/tree) and replicated hi-sums S1r
     (3r x 4 combos/tree) via constant 0/1 selection matmuls.
  5. ACT: E2 = exp(S2), E1r = exp(S1r).
  6. PE: M1[t,(r,hi)] = sum_lo resp[t,hi*16+lo,r] * E2[t,lo]  (block-diag).
  7. DVE: P = M1 * E1r.
  8. PE: out[t*3+r] = sum_hi P, accumulated 4 groups per psum via
     zero-column selection weights.
Generic path (any path_map): 2-trees-per-matmul leaf log-sum (64 leaves),
exp, response block-diag accumulation.
All matmul operands are float32r (FP22 single-pass PE mode).
"""
import numpy as np
from contextlib import ExitStack

import concourse.bass as bass
import concourse.bacc as bacc
import concourse.tile as tile
import concourse.mybir as mybir
from concourse.bass_utils import run_bass_kernel_spmd

F32 = mybir.dt.float32
F32R = mybir.dt.float32r
AF = mybir.ActivationFunctionType
ALU = mybir.AluOpType

B = 1024          # batch
F = 512           # in_features
T = 512           # num_trees
D = 6             # depth
R = 3             # response_dim
NLEAF = 64
NCORES = 8
T_C = T // NCORES          # 64 trees per core
TPG = 8                    # trees per gate-tile group
NG = T_C // TPG            # 8 groups per core
MROW = 64                  # padded rows per fv M-tile (48 real + 16 pad)
NPAIR = T_C // 2           # generic path: 32 tree-pairs per core
PAIRS_PER_EG = 16
EPS = 2.0 ** -20
NH = 2                     # N halves (1024 = 2 x 512)
NLO = 16                   # 2^4 lo-combos (depths 0..3)
NHI = 4                    # 2^2 hi-combos (depths 4..5)

_CACHE = {}


def _is_oblivious(path_map):
    pm = np.asarray(path_map).reshape(NLEAF, D)
    exp = np.array([[2 * j + ((l >> j) & 1) for j in range(D)]
                    for l in range(NLEAF)], dtype=pm.dtype)
    return bool(np.array_equal(pm, exp))


def _gate_row(t_loc, g):
    """pg-tile row of gate g (= 2d+s) for local tree t_loc."""
    d, s = g // 2, g % 2
    return (64 if s else 0) + 6 * t_loc + d


# ───────────────────────── fast (oblivious) constants ─────────────────────

def _build_sel2c():
    """[128, 128] lo-sum selection: col = 16*t_loc + lo, depths 0..3."""
    S = np.zeros((128, 128), np.float32)
    for t_loc in range(TPG):
        for lo in range(NLO):
            col = NLO * t_loc + lo
            for j in range(4):
                S[_gate_row(t_loc, 2 * j + ((lo >> j) & 1)), col] = 1.0
    return S


def _build_sel1r():
    """[128, 96] replicated hi-sum selection: col = 12*t_loc + 4*r + hi."""
    S = np.zeros((128, 96), np.float32)
    for t_loc in range(TPG):
        for r in range(R):
            for hi in range(NHI):
                col = 12 * t_loc + 4 * r + hi
                for j in range(4, 6):
                    S[_gate_row(t_loc, 2 * j + ((hi >> (j - 4)) & 1)), col] = 1.0
    return S


def _build_selh():
    """[96, 4*96] hi-reduce: 4 variants (group slot in psum accumulation).

    variant v: rows = P rows (12*t_loc + 4*r + hi), col = 24*v + 3*t_loc + r.
    """
    S = np.zeros((96, 4 * 96), np.float32)
    for v in range(4):
        for t_loc in range(TPG):
            for r in range(R):
                for hi in range(NHI):
                    S[12 * t_loc + 4 * r + hi, 96 * v + 24 * v + 3 * t_loc + r] = 1.0
    return S


def _build_resp2(response_core):
    """[128, NG*96]: per group g, rows 16*t_loc+lo, col 12*t_loc+4*r+hi =
    response[8g+t_loc, hi*16+lo, r]."""
    out = np.zeros((128, NG * 96), np.float32)
    for g in range(NG):
        for t_loc in range(TPG):
            t = TPG * g + t_loc
            for hi in range(NHI):
                for r in range(R):
                    out[NLO * t_loc:NLO * t_loc + NLO,
                        96 * g + 12 * t_loc + 4 * r + hi] = \
                        response_core[t, hi * NLO:(hi + 1) * NLO, r]
    return out


# ───────────────────────── generic-path constants ─────────────────────────

def _build_sel_generic(path_map):
    pm = np.asarray(path_map).reshape(NLEAF, D)
    sel = np.zeros((4, 128, 128), np.float32)
    for k in range(4):
        for t01 in range(2):
            t_loc = 2 * k + t01
            for leaf in range(NLEAF):
                col = 64 * t01 + leaf
                for j in range(D):
                    sel[k, _gate_row(t_loc, int(pm[leaf, j])), col] += 1.0
    return np.ascontiguousarray(sel.transpose(1, 0, 2).reshape(128, 512))


def _build_rbd_generic(response_core):
    rbd = np.zeros((128, NPAIR * 96), np.float32)
    for p in range(NPAIR):
        q = p % PAIRS_PER_EG
        for t01 in range(2):
            t = 2 * p + t01
            c0 = 96 * p + 6 * q + 3 * t01
            rbd[64 * t01:64 * t01 + 64, c0:c0 + 3] = response_core[t]
    return rbd


# ───────────────────────── program builders ──────────────────────────────

def _patched_act_tables():
    """Force Ln+Exp onto the shared natural_log_exp_and_others table set
    so the ACT LUT isn't reloaded between ln and exp phases."""
    import concourse.bacc as bacc_mod
    from concourse.hw_specs import get_activation_tables as orig

    def patched(arch):
        tabs = orig(arch)
        if "natural_log_exp_and_others" in tabs:
            for name, funcs in tabs.items():
                if name != "natural_log_exp_and_others":
                    funcs.discard(AF.Ln)
                    funcs.discard(AF.Exp)
        return tabs

    class _Ctx:
        def __enter__(self):
            self.saved = bacc_mod.get_activation_tables
            bacc_mod.get_activation_tables = patched

        def __exit__(self, *a):
            bacc_mod.get_activation_tables = self.saved

    return _Ctx()


def _common_frontend(nc, tc, ctx):
    """DMA inputs and ecw = exp(feat_attention) tiles."""
    xt = nc.dram_tensor("xt", [F, B + 2], F32R, kind="ExternalInput")
    fap = nc.dram_tensor("fap", [F, NG * MROW], F32, kind="ExternalInput")
    ta0 = nc.dram_tensor("ta0", [128, 4], F32, kind="ExternalInput")
    tbb = nc.dram_tensor("tbb", [128, 4], F32, kind="ExternalInput")

    cpool = ctx.enter_context(tc.tile_pool(name="consts", bufs=1))
    txt = [cpool.tile([128, B + 2], F32R, name=f"txt{k}", tag=f"xt{k}")
           for k in range(4)]
    tfap = [cpool.tile([128, NG * MROW], F32, name=f"tfap{k}", tag=f"fap{k}")
            for k in range(4)]
    tecw = [cpool.tile([128, NG * MROW], F32R, name=f"tecw{k}", tag=f"ecw{k}")
            for k in range(4)]
    tta0 = cpool.tile([128, 4], F32)
    ttb = cpool.tile([128, 4], F32)
    tra = cpool.tile([128, 4], F32)
    trz = cpool.tile([128, 8], F32)

    for k in range(4):
        nc.sync.dma_start(txt[k][:], xt[128 * k:128 * k + 128, :])
        nc.sync.dma_start(tfap[k][:], fap[128 * k:128 * k + 128, :])
    nc.sync.dma_start(tta0[:], ta0[:])
    nc.sync.dma_start(ttb[:], tbb[:])

    for k in range(4):
        nc.scalar.activation(tecw[k][:], tfap[k][:], AF.Exp)

    return cpool, txt, tecw, tra, trz, tta0, ttb


def _emit_glogs(nc, tc, ctx, txt, tecw, tra, trz, tta0, ttb, glog_bufs):
    """Per-group gate-log tiles via M=128 fv matmuls with fused Z columns.
    Returns list of glog APs."""
    glpool = ctx.enter_context(tc.tile_pool(name="glp", bufs=glog_bufs))
    lctx = ctx.enter_context(ExitStack())
    fvpool = lctx.enter_context(tc.tile_pool(name="fvps", bufs=1, space="PSUM"))
    wpool = lctx.enter_context(tc.tile_pool(name="work", bufs=2))
    pgpool = lctx.enter_context(tc.tile_pool(name="pgp", bufs=2))
    glogs = [None] * NG
    for m in range(4):          # M-tile = 2 gate groups (2m, 2m+1)
        fv = fvpool.tile([128, B + 2], F32, name=f"fv{m}", tag="fv")
        for k in range(4):
            for off, n in ((0, 512), (512, 512), (1024, 2)):
                nc.tensor.matmul(fv[:, off:off + n],
                                 tecw[k][:, 128 * m:128 * (m + 1)],
                                 txt[k][:, off:off + n],
                                 start=(k == 0), stop=(k == 3))
        nc.vector.reciprocal(trz[:, 2 * m:2 * m + 2], fv[:, 1024:1026])
        nc.vector.tensor_mul(tra[:, m:m + 1], tta0[:, m:m + 1],
                             trz[:, 2 * m:2 * m + 1])
        tmp = wpool.tile([128, B], F32, name=f"tmp{m}", tag="tmp")
        nc.vector.tensor_scalar(tmp[:], fv[:, 0:1024], tra[:, m:m + 1],
                                ttb[:, m:m + 1], ALU.mult, ALU.add)
        for half in range(2):
            g = 2 * m + half
            th = tmp[64 * half:64 * half + 64, :]
            pg = pgpool.tile([128, B], F32R, name=f"pg{g}", tag="pg")
            nc.gpsimd.tensor_scalar(pg[0:64, :], th, 1.0, EPS, ALU.min, ALU.max)
            nc.gpsimd.tensor_scalar(pg[64:128, :], th, -1.0, 1.0,
                                    ALU.mult, ALU.add)
            nc.vector.tensor_scalar(pg[64:128, :], pg[64:128, :], 1.0 - EPS,
                                    EPS, ALU.min, ALU.max)
            glog = glpool.tile([128, B], F32R, name=f"glog{g}", tag="glog")
            nc.scalar.activation(glog[:], pg[:], AF.Ln)
            glogs[g] = glog
    lctx.close()
    return glogs


def _build_program_fast():
    nc = bacc.Bacc("TRN2", target_bir_lowering=False, debug=False,
                   num_devices=NCORES)
    sel2c = nc.dram_tensor("sel2c", [128, 128], F32R, kind="ExternalInput")
    sel1r = nc.dram_tensor("sel1r", [128, 96], F32R, kind="ExternalInput")
    selh = nc.dram_tensor("selh", [96, 4 * 96], F32R, kind="ExternalInput")
    resp2 = nc.dram_tensor("resp2", [128, NG * 96], F32R, kind="ExternalInput")
    out = nc.dram_tensor("out", [T_C * R, B], F32, kind="ExternalOutput")

    with tile.TileContext(nc) as tc, ExitStack() as ctx:
        cpool, txt, tecw, tra, trz, tta0, ttb = _common_frontend(nc, tc, ctx)
        tsel2c = cpool.tile([128, 128], F32R)
        tsel1r = cpool.tile([128, 96], F32R)
        tselh = cpool.tile([96, 4 * 96], F32R)
        tresp2 = cpool.tile([128, NG * 96], F32R)
        nc.sync.dma_start(tsel2c[:], sel2c[:])
        nc.sync.dma_start(tsel1r[:], sel1r[:])
        nc.sync.dma_start(tselh[:], selh[:])
        nc.sync.dma_start(tresp2[:], resp2[:])

        glogs = _emit_glogs(nc, tc, ctx, txt, tecw, tra, trz, tta0, ttb,
                            glog_bufs=NG)

        e2pool = ctx.enter_context(tc.tile_pool(name="e2p", bufs=NG))
        e1pool = ctx.enter_context(tc.tile_pool(name="e1p", bufs=NG))
        ppool = ctx.enter_context(tc.tile_pool(name="pp", bufs=3))
        evpool = ctx.enter_context(tc.tile_pool(name="evp", bufs=2))

        # lo- and hi-sums + exp, one pass per group (pools coexist: 8 banks)
        e2s, e1s = [], []
        with (
            tc.tile_pool(name="s2ps", bufs=2, space="PSUM") as s2pool,
            tc.tile_pool(name="s1ps", bufs=2, space="PSUM") as s1pool,
        ):
            for g in range(NG):
                s2 = s2pool.tile([128, B], F32, name=f"s2_{g}", tag="s2")
                for nh in range(NH):
                    nc.tensor.matmul(s2[:, 512 * nh:512 * (nh + 1)], tsel2c[:],
                                     glogs[g][:, 512 * nh:512 * (nh + 1)],
                                     start=True, stop=True)
                e2 = e2pool.tile([128, B], F32R, name=f"e2_{g}", tag="e2")
                nc.scalar.activation(e2[:], s2[:], AF.Exp)
                e2s.append(e2)
                s1 = s1pool.tile([96, B], F32, name=f"s1_{g}", tag="s1")
                for nh in range(NH):
                    nc.tensor.matmul(s1[:, 512 * nh:512 * (nh + 1)], tsel1r[:],
                                     glogs[g][:, 512 * nh:512 * (nh + 1)],
                                     start=True, stop=True)
                e1 = e1pool.tile([96, B], F32R, name=f"e1_{g}", tag="e1")
                nc.scalar.activation(e1[:], s1[:], AF.Exp)
                e1s.append(e1)
        # response contraction + hi reduce
        with (
            tc.tile_pool(name="m1ps", bufs=2, space="PSUM") as m1pool,
            tc.tile_pool(name="ops", bufs=1, space="PSUM") as opool,
        ):
            for eg in range(2):
                op = opool.tile([96, B], F32, name=f"op{eg}", tag="outp")
                for v in range(4):
                    g = 4 * eg + v
                    m1 = m1pool.tile([96, B], F32, name=f"m1_{g}", tag="m1")
                    for nh in range(NH):
                        nc.tensor.matmul(m1[:, 512 * nh:512 * (nh + 1)],
                                         tresp2[:, 96 * g:96 * (g + 1)],
                                         e2s[g][:, 512 * nh:512 * (nh + 1)],
                                         start=True, stop=True)
                    pp = ppool.tile([96, B], F32R, name=f"pp{g}", tag="pp")
                    nc.vector.tensor_mul(pp[:], m1[:], e1s[g][:])
                    for nh in range(NH):
                        nc.tensor.matmul(op[:, 512 * nh:512 * (nh + 1)],
                                         tselh[:, 96 * v:96 * (v + 1)],
                                         pp[:, 512 * nh:512 * (nh + 1)],
                                         start=(v == 0), stop=(v == 3),
                                         skip_group_check=True)
                ev = evpool.tile([96, B], F32, name=f"ev{eg}", tag="ev")
                nc.vector.tensor_copy(ev[:], op[:])
                nc.sync.dma_start(out[96 * eg:96 * (eg + 1), :], ev[:])

    with _patched_act_tables():
        nc.compile()
    return nc


def _build_program_generic():
    nc = bacc.Bacc("TRN2", target_bir_lowering=False, debug=False,
                   num_devices=NCORES)
    selz = nc.dram_tensor("selz", [128, 512], F32R, kind="ExternalInput")
    rbd = nc.dram_tensor("rbd", [128, NPAIR * 96], F32R, kind="ExternalInput")
    out = nc.dram_tensor("out", [T_C * R, B], F32, kind="ExternalOutput")

    with tile.TileContext(nc) as tc, ExitStack() as ctx:
        cpool, txt, tecw, tra, trz, tta0, ttb = _common_frontend(nc, tc, ctx)
        tselz = cpool.tile([128, 512], F32R)
        trbd = cpool.tile([128, NPAIR * 96], F32R)
        nc.sync.dma_start(tselz[:], selz[:])
        nc.sync.dma_start(trbd[:], rbd[:])

        glogs = _emit_glogs(nc, tc, ctx, txt, tecw, tra, trz, tta0, ttb,
                            glog_bufs=3)

        rwpool = ctx.enter_context(tc.tile_pool(name="rwp", bufs=3))
        evpool = ctx.enter_context(tc.tile_pool(name="evp", bufs=2))
        with (
            tc.tile_pool(name="sps", bufs=2, space="PSUM") as spool,
            tc.tile_pool(name="ops", bufs=1, space="PSUM") as opool,
        ):
            for eg in range(2):
                op = opool.tile([96, B], F32, name=f"op{eg}", tag="outp")
                for gi in range(NG // 2):
                    g = eg * (NG // 2) + gi
                    for k in range(4):
                        p = 4 * g + k
                        q = p % PAIRS_PER_EG
                        sp = spool.tile([128, B], F32, name=f"sp{p}", tag="s")
                        for nh in range(NH):
                            nc.tensor.matmul(sp[:, 512 * nh:512 * (nh + 1)],
                                             tselz[:, 128 * k:128 * (k + 1)],
                                             glogs[g][:, 512 * nh:512 * (nh + 1)],
                                             start=True, stop=True)
                        rw = rwpool.tile([128, B], F32R, name=f"rw{p}", tag="rw")
                        nc.scalar.activation(rw[:], sp[:], AF.Exp)
                        for nh in range(NH):
                            nc.tensor.matmul(op[:, 512 * nh:512 * (nh + 1)],
                                             trbd[:, 96 * p:96 * (p + 1)],
                                             rw[:, 512 * nh:512 * (nh + 1)],
                                             start=(q == 0),
                                             stop=(q == PAIRS_PER_EG - 1),
                                             skip_group_check=True)
                ev = evpool.tile([96, B], F32, name=f"ev{eg}", tag="ev")
                nc.vector.tensor_copy(ev[:], op[:])
                nc.sync.dma_start(out[96 * eg:96 * (eg + 1), :], ev[:])

    with _patched_act_tables():
        nc.compile()
    return nc


# ───────────────────────── host entry point ──────────────────────────────

def _host_prep_core(c, xto, feat_attention, a0_all, b_all):
    t0 = T_C * c
    fa_c = feat_attention[:, D * t0: D * (t0 + T_C)]
    fap = np.zeros((F, NG * MROW), np.float32)
    ta0 = np.zeros((128, 4), np.float32)
    tbb = np.full((128, 4), 0.5, np.float32)
    for g in range(NG):
        fap[:, MROW * g: MROW * g + 48] = fa_c[:, 48 * g: 48 * g + 48]
        m, half = g // 2, g % 2
        for t_loc in range(TPG):
            t = t0 + TPG * g + t_loc
            rows = slice(64 * half + 6 * t_loc, 64 * half + 6 * t_loc + 6)
            ta0[rows, m] = a0_all[t]
            tbb[rows, m] = b_all[t]
    return dict(xt=xto, fap=fap, ta0=ta0, tbb=tbb)


def kernel(x, feat_attention, thresholds, log_temperatures, response, path_map):
    x = np.ascontiguousarray(np.asarray(x, dtype=np.float32))
    feat_attention = np.asarray(feat_attention, dtype=np.float32)
    thresholds = np.asarray(thresholds, dtype=np.float32)
    log_temperatures = np.asarray(log_temperatures, dtype=np.float32)
    response = np.asarray(response, dtype=np.float32)

    fast = _is_oblivious(path_map)
    key = "fast" if fast else "generic"
    if key not in _CACHE:
        _CACHE[key] = (_build_program_fast() if fast
                       else _build_program_generic())
    nc = _CACHE[key]

    xto = np.ascontiguousarray(
        np.concatenate([x.T, np.ones((F, 2), np.float32)], axis=1))
    elt = np.exp(-log_temperatures)
    a0_all = 0.5 * elt
    b_all = 0.5 - 0.5 * thresholds * elt

    in_maps = []
    for c in range(NCORES):
        m = _host_prep_core(c, xto, feat_attention, a0_all, b_all)
        t0 = T_C * c
        if fast:
            m["sel2c"] = _CACHE.setdefault("sel2c", _build_sel2c())
            m["sel1r"] = _CACHE.setdefault("sel1r", _build_sel1r())
            m["selh"] = _CACHE.setdefault("selh", _build_selh())
            m["resp2"] = _build_resp2(response[t0:t0 + T_C])
        else:
            if "selg" not in _CACHE:
                _CACHE["selg"] = _build_sel_generic(path_map)
            m["selz"] = _CACHE["selg"]
            m["rbd"] = _build_rbd_generic(response[t0:t0 + T_C])
        in_maps.append(m)

    _CACHE["in_maps"] = in_maps
    res = run_bass_kernel_spmd(nc, in_maps, core_ids=list(range(NCORES)))
    outs = [res.results[c]["out"].T for c in range(NCORES)]
    return np.ascontiguousarray(np.concatenate(outs, axis=1))
